# revision 14
# baseline (speedup 1.0000x reference)
"""Trainium2 Bass kernel for CrossAttention (B=2, N=2048, C=768, H=12).

Sharding: core c -> batch b=c//4, head-group g=c%4 (3 heads each).
Each core computes Q/K/V projections for its heads over the full sequence and
attention; an AllToAll exchanges per-head outputs so each core then computes
the full output projection, residual and LayerNorm for its own 512-row
q-shard.

v2 schedule: single fused region.  K-proj warms the PE, Q-proj is emitted
just-in-time per 512-column q-chunk and V-proj just-in-time per kv-block so
projection matmuls fill the PE bubbles of the scalar(exp)-bound attention
loop (keeps the HAM clock-gate at 8/8).  The AllToAll is split in two:
heads {0,1} fire after their attention finishes and transfer under head-2's
attention; only the small head-2 AllToAll plus the tail of the output
projection is exposed.

kernel(**inputs) takes the FULL inputs (setup_inputs() keys) and returns the
full [2, 2048, 768] output.
"""

import sys

for _p in ("/opt/trn_rl_repo",):
    if _p not in sys.path:
        sys.path.insert(0, _p)

import numpy as np

B, N, C = 2, 2048, 768
H = 12
DH = 64
EPS = 1e-5
SCALE = DH ** (-0.5)  # 0.125

NCORES = 8
HPC = 3          # heads per core
CS = HPC * DH    # 192 output-feature slice per core
QS = N // 4      # 512 q rows per core
P = 128

_NC_CACHE = {}

# Wo row permutation: gathered order is [per-group heads (3g, 3g+1)] then
# [per-group head 3g+2]; Wo rows must match.
import numpy as _np
WO_PERM = _np.concatenate(
    [_np.arange(192 * g, 192 * g + 128) for g in range(4)]
    + [_np.arange(192 * g + 128, 192 * (g + 1)) for g in range(4)]
)


def _build_nc():
    import concourse.bass as bass
    import concourse.mybir as mybir
    import concourse.tile as tile
    from concourse import bacc

    f32 = mybir.dt.float32
    bf16 = mybir.dt.bfloat16
    Alu = mybir.AluOpType
    Act = mybir.ActivationFunctionType

    nc = bacc.Bacc(
        "TRN2",
        target_bir_lowering=False,
        debug=False,
        enable_asserts=False,
        num_devices=NCORES,
    )

    # ---- kernel I/O (per-core shapes; host shards the full problem) ----
    qT = nc.dram_tensor("qT", [C, N], bf16, kind="ExternalInput").ap()
    kT = nc.dram_tensor("kT", [C, N], bf16, kind="ExternalInput").ap()
    vT = nc.dram_tensor("vT", [C, N], bf16, kind="ExternalInput").ap()
    wq = nc.dram_tensor("wq", [C, CS], bf16, kind="ExternalInput").ap()
    wk = nc.dram_tensor("wk", [C, CS], bf16, kind="ExternalInput").ap()
    wv = nc.dram_tensor("wv", [C, CS], bf16, kind="ExternalInput").ap()
    wo = nc.dram_tensor("wo", [C, C], bf16, kind="ExternalInput").ap()
    bq = nc.dram_tensor("bq", [CS], f32, kind="ExternalInput").ap()
    bk = nc.dram_tensor("bk", [CS], f32, kind="ExternalInput").ap()
    bv = nc.dram_tensor("bv", [CS], f32, kind="ExternalInput").ap()
    gamma = nc.dram_tensor("gamma", [C], f32, kind="ExternalInput").ap()
    beta = nc.dram_tensor("beta", [C], f32, kind="ExternalInput").ap()
    qres = nc.dram_tensor("qres", [QS, C], f32, kind="ExternalInput").ap()
    gsel = nc.dram_tensor("gsel", [2], f32, kind="ExternalInput").ap()
    y = nc.dram_tensor("y", [QS, C], f32, kind="ExternalOutput").ap()

    CI = C // P          # 6 contraction chunks
    NJ = N // 512        # 4 q-chunks of 512
    NM = N // P          # 16 kv-chunks of 128
    VS = DH + 1          # 65: v columns + ones column (denominator row)
    QT = QS // P         # 4 output row-blocks of 128

    with tile.TileContext(nc) as tc:
        const = tc.alloc_tile_pool(name="const", bufs=1)
        persist = tc.alloc_tile_pool(name="persist", bufs=1)
        rows = tc.alloc_tile_pool(name="rows", bufs=2)
        ppool = tc.alloc_tile_pool(name="ppool", bufs=3)
        small = tc.alloc_tile_pool(name="small", bufs=4)
        dram = tc.alloc_tile_pool(name="dram", bufs=1, space="DRAM")

        # ---------- weights + j-blocked input DMAs (priority order) -------
        wk_sb = const.tile([P, CI, CS], bf16, name="wk_sb")
        nc.sync.dma_start(wk_sb[:], wk.rearrange("(o p) m -> p o m", p=P))
        bkA = const.tile([P, 1], f32, name="bkA")
        bkB = const.tile([DH, 1], f32, name="bkB")
        nc.sync.dma_start(bkA[:], bk[0:P][:, None])
        nc.sync.dma_start(bkB[:], bk[P:CS][:, None])
        wq_sb = const.tile([P, CI, CS], bf16, name="wq_sb")
        nc.sync.dma_start(wq_sb[:], wq.rearrange("(o p) m -> p o m", p=P))
        bqA = const.tile([P, 1], f32, name="bqA")
        bqB = const.tile([DH, 1], f32, name="bqB")
        nc.sync.dma_start(bqA[:], bq[0:P][:, None])
        nc.sync.dma_start(bqB[:], bq[P:CS][:, None])
        wv_sb = const.tile([P, CI, CS], bf16, name="wv_sb")
        nc.sync.dma_start(wv_sb[:], wv.rearrange("(o p) m -> p o m", p=P))
        bv_b = const.tile([P, CS], f32, name="bv_b")
        nc.sync.dma_start(bv_b[0:1, :], bv[None, :])
        nc.gpsimd.partition_broadcast(bv_b[:], bv_b[0:1, :])

        k_rows = [
            rows.tile([P, N], bf16, tag="krow", bufs=6, name=f"k_row{i}")
            for i in range(CI)
        ]
        q_rows = [persist.tile([P, N], bf16, name=f"q_row{i}") for i in range(CI)]
        v_rows = [
            rows.tile([P, N], bf16, tag="vrow", bufs=6, name=f"v_row{i}")
            for i in range(CI)
        ]
        # j-block 0 of kT, then q columns 0:512, then vT block 0, then the
        # rest round-robin so compute can chase the DMA stream
        for i in range(CI):
            nc.sync.dma_start(k_rows[i][:, 0:512], kT[P * i : P * (i + 1), 0:512])
        for i in range(CI):
            nc.sync.dma_start(q_rows[i][:, 0:512], qT[P * i : P * (i + 1), 0:512])
        for i in range(CI):
            nc.sync.dma_start(v_rows[i][:, 0:512], vT[P * i : P * (i + 1), 0:512])
        for j in range(1, NJ):
            s5 = slice(512 * j, 512 * (j + 1))
            for i in range(CI):
                nc.sync.dma_start(k_rows[i][:, s5], kT[P * i : P * (i + 1), s5])
            for i in range(CI):
                nc.sync.dma_start(v_rows[i][:, s5], vT[P * i : P * (i + 1), s5])
        for i in range(CI):
            nc.sync.dma_start(q_rows[i][:, 512:N], qT[P * i : P * (i + 1), 512:N])

        # ---------- persistent activations ----------
        qTa = persist.tile([P, N], bf16, name="qTa")    # heads 0,1
        qTb = persist.tile([DH, N], bf16, name="qTb")   # head 2
        kTa = persist.tile([P, N], bf16, name="kTa")
        kTb = persist.tile([DH, N], bf16, name="kTb")
        vaug = persist.tile([P, NM, HPC * VS], bf16, name="vaug")
        nc.vector.memset(
            vaug.rearrange("p m (h d) -> p m h d", d=VS)[:, :, :, DH : DH + 1], 1.0
        )
        o_hb = [persist.tile([DH, N], bf16, name=f"ob{h}") for h in range(HPC)]
        oG = persist.tile([P, CI, QS], bf16, name="oG")

        wo_sb = const.tile([P, CI, C], bf16, name="wo_sb")
        gs = const.tile([1, 2], f32, name="gs")
        s0b = const.tile([P, 1], f32, name="s0b")
        s1b = const.tile([P, 1], f32, name="s1b")
        gamma_b = const.tile([P, C], f32, name="gamma_b")
        beta_b = const.tile([P, C], f32, name="beta_b")
        qres_sb = const.tile([P, QT, C], f32, name="qres_sb")

        a2a1_in = dram.tile([2 * NJ, P, QS], bf16, name="a2a1_in")
        a2a1_out = dram.tile([2 * NJ, P, QS], bf16, name="a2a1_out")
        a2a2_in = dram.tile([2 * NJ, DH, QS], bf16, name="a2a2_in")
        a2a2_out = dram.tile([2 * NJ, DH, QS], bf16, name="a2a2_out")

        # =========== fused projections + attention (heads 0,1) ===========
        with (
            tc.tile_pool(name="ppO", bufs=1, space="PSUM") as ppO,
            tc.tile_pool(name="ppS", bufs=2, space="PSUM") as ppS,
            tc.tile_pool(name="ppF", bufs=2, space="PSUM") as ppF,
        ):
            po_h = {
                0: ppO.tile([P, 512], f32, tag="po0", name="po0"),
                1: ppO.tile([P, 512], f32, tag="po1", name="po1"),
            }

            def k_proj(j):
                s5 = slice(512 * j, 512 * (j + 1))
                pk_a = ppF.tile([P, 512], f32, tag="fill", name=f"pka{j}")
                for i in range(CI):
                    nc.tensor.matmul(
                        pk_a[:], wk_sb[:, i, 0:P], k_rows[i][:, s5],
                        start=(i == 0), stop=(i == CI - 1),
                    )
                nc.vector.tensor_tensor(
                    kTa[:, s5], pk_a[:], bkA.to_broadcast((P, 512)), Alu.add
                )
                pk_b = ppF.tile([P, 512], f32, tag="fill", name=f"pkb{j}")
                for i in range(CI):
                    nc.tensor.matmul(
                        pk_b[0:DH], wk_sb[:, i, P:CS], k_rows[i][:, s5],
                        start=(i == 0), stop=(i == CI - 1),
                    )
                nc.vector.tensor_tensor(
                    kTb[:, s5], pk_b[0:DH], bkB.to_broadcast((DH, 512)), Alu.add
                )

            q_state = {}

            def q_proj_piece(r, piece):
                """Emit 2 of the 12 Q-proj matmuls for chunk r (piece 0..5)."""
                s5 = slice(512 * r, 512 * (r + 1))
                if piece < 3:
                    if piece == 0:
                        q_state[r] = ppF.tile(
                            [P, 512], f32, tag="fill", name=f"pqa{r}"
                        )
                    pq = q_state[r]
                    for i in (2 * piece, 2 * piece + 1):
                        nc.tensor.matmul(
                            pq[:], wq_sb[:, i, 0:P], q_rows[i][:, s5],
                            start=(i == 0), stop=(i == CI - 1),
                        )
                    if piece == 2:
                        nc.vector.tensor_tensor(
                            qTa[:, s5], pq[:], bqA.to_broadcast((P, 512)), Alu.add
                        )
                else:
                    if piece == 3:
                        q_state[r] = ppF.tile(
                            [P, 512], f32, tag="fill", name=f"pqb{r}"
                        )
                    pq = q_state[r]
                    for i in (2 * (piece - 3), 2 * (piece - 3) + 1):
                        nc.tensor.matmul(
                            pq[0:DH], wq_sb[:, i, P:CS], q_rows[i][:, s5],
                            start=(i == 0), stop=(i == CI - 1),
                        )
                    if piece == 5:
                        nc.vector.tensor_tensor(
                            qTb[:, s5], pq[0:DH], bqB.to_broadcast((DH, 512)), Alu.add
                        )

            def q_proj(r):
                for piece in range(6):
                    q_proj_piece(r, piece)

            def v_proj(m):
                pv = ppF.tile([P, 512], f32, tag="fill", name=f"pv{m}")
                for i in range(CI):
                    nc.tensor.matmul(
                        pv[:, 0:CS], v_rows[i][:, P * m : P * (m + 1)], wv_sb[:, i, :],
                        start=(i == 0), stop=(i == CI - 1),
                    )
                dst = vaug.rearrange("p m (h d) -> p m h d", d=VS)[:, m, :, 0:DH]
                nc.vector.tensor_tensor(
                    dst,
                    pv[:, 0:CS].rearrange("p (h d) -> p h d", d=DH),
                    bv_b.rearrange("p (h d) -> p h d", d=DH),
                    Alu.add,
                )

            def evict_head(h, r, po):
                s5 = slice(512 * r, 512 * (r + 1))
                l_t = small.tile([1, 512], f32, tag="lt", name=f"l{h}{r}")
                nc.vector.tensor_copy(l_t[:], po[DH : DH + 1, :])
                r_t = small.tile([1, 512], f32, tag="lt", name=f"rr{h}{r}")
                nc.vector.reciprocal_approx_fast(out=r_t[:], in_=l_t[:])
                rb = ppool.tile([DH, 512], f32, tag="rb", bufs=2, name=f"rb{h}{r}")
                nc.gpsimd.partition_broadcast(rb[:], r_t[:])
                nc.vector.tensor_tensor(o_hb[h][:, s5], po[0:DH, :], rb[:], Alu.mult)

            # projection lead-in: K j=0, Q r=0 chase the first DMA blocks
            k_proj(0)
            q_proj(0)
            v_proj(0)
            v_proj(1)

            # flat software-pipelined loop over (r, m); av trails by 1 step
            steps = [(r, m) for r in range(NJ) for m in range(NM)]
            pts = {}
            for si, (r, m) in enumerate(steps):
                sq = slice(512 * r, 512 * (r + 1))
                sm = slice(P * m, P * (m + 1))
                s_t = ppS.tile([P, 1024], f32, tag="s", name=f"s{r}_{m}")
                nc.tensor.matmul(
                    s_t[:, 0:512], kTa[0:DH, sm], qTa[0:DH, sq],
                    start=True, stop=True,
                )
                nc.tensor.matmul(
                    s_t[:, 512:1024], kTa[DH:P, sm], qTa[DH:P, sq],
                    start=True, stop=True,
                )
                pt = ppool.tile([P, 1024], bf16, tag="p", bufs=6, name="pt")
                nc.scalar.activation(pt[:], s_t[:], Act.Exp, scale=SCALE)
                pts[si] = pt
                # PE fillers while exp runs: rest of K-proj, V-proj, Q-proj
                if r == 0:
                    if m in (1, 5, 9):
                        k_proj(m // 4 + 1)
                    if m + 2 < NM:
                        v_proj(m + 2)
                if r < NJ - 1:
                    if r == 0 and m == 14:
                        q_proj(1)
                    elif r >= 1 and 6 <= m <= 11:
                        q_proj_piece(r + 1, m - 6)
                if si >= 1:
                    pr, pm = steps[si - 1]
                    for h in (0, 1):
                        nc.tensor.matmul(
                            po_h[h][0:VS],
                            vaug[:, pm, VS * h : VS * (h + 1)],
                            pts[si - 1][:, 512 * h : 512 * (h + 1)],
                            start=(pm == 0), stop=(pm == NM - 1),
                        )
                    del pts[si - 1]
                    if pm == NM - 1:
                        for h in (0, 1):
                            evict_head(h, pr, po_h[h])
                        for h in (0, 1):
                            for g2 in range(2):
                                nc.sync.dma_start(
                                    a2a1_in[NJ * g2 + pr, DH * h : DH * (h + 1), :],
                                    o_hb[h][:, 512 * pr : 512 * (pr + 1)],
                                )
            (r, m) = steps[-1]
            for h in (0, 1):
                nc.tensor.matmul(
                    po_h[h][0:VS],
                    vaug[:, m, VS * h : VS * (h + 1)],
                    pts[len(steps) - 1][:, 512 * h : 512 * (h + 1)],
                    start=False, stop=True,
                )
            for h in (0, 1):
                evict_head(h, r, po_h[h])
            for h in (0, 1):
                for g2 in range(2):
                    nc.sync.dma_start(
                        a2a1_in[NJ * g2 + r, DH * h : DH * (h + 1), :],
                        o_hb[h][:, 512 * r : 512 * (r + 1)],
                    )

        nc.gpsimd.collective_compute(
            "AllToAll",
            Alu.bypass,
            replica_groups=[list(range(NCORES))],
            ins=[a2a1_in.opt()],
            outs=[a2a1_out.opt()],
        )

        # tail-only constants (kept off the startup DMA queues)
        nc.sync.dma_start(wo_sb[:], wo.rearrange("(o p) m -> p o m", p=P))
        nc.sync.dma_start(gs[:], gsel[None, :])
        nc.gpsimd.partition_broadcast(s0b[:], gs[0:1, 0:1])
        nc.gpsimd.partition_broadcast(s1b[:], gs[0:1, 1:2])
        nc.sync.dma_start(gamma_b[0:1, :], gamma[None, :])
        nc.sync.dma_start(beta_b[0:1, :], beta[None, :])
        nc.gpsimd.partition_broadcast(gamma_b[:], gamma_b[0:1, :])
        nc.gpsimd.partition_broadcast(beta_b[:], beta_b[0:1, :])
        nc.sync.dma_start(qres_sb[:], qres.rearrange("(t p) c -> p t c", p=P))

        # ------- head 2: own PSUM scope, per-r accumulators, evicts -------
        # deferred so nothing in the attention pipeline waits on the gpsimd
        # queue (which is blocked by the heads-0,1 collective wait)
        with (
            tc.tile_pool(name="ppS2", bufs=2, space="PSUM") as ppS2,
            tc.tile_pool(name="ppO2", bufs=1, space="PSUM") as ppO2,
        ):
            h2 = 2
            po2s = [
                ppO2.tile([P, 512], f32, tag=f"po2_{r}", name=f"po2_{r}")
                for r in range(NJ)
            ]
            # q-chunk pairs (2rp, 2rp+1) share every stationary operand, so
            # each LDWEIGHTS feeds two back-to-back matmuls
            h2_steps = [(rp, mp) for rp in range(NJ // 2) for mp in range(NM // 2)]
            pt2s = {}

            def h2_avs(si):
                prp, pmp = h2_steps[si]
                ptA, ptB = pt2s[si]
                for q2 in range(2):
                    m = 2 * pmp + q2
                    for ri, pt in ((0, ptA), (1, ptB)):
                        nc.tensor.matmul(
                            po2s[2 * prp + ri][0:VS],
                            vaug[:, m, VS * h2 : VS * (h2 + 1)],
                            pt[:, 512 * q2 : 512 * (q2 + 1)],
                            start=(m == 0), stop=(m == NM - 1),
                        )
                del pt2s[si]

            for si, (rp, mp) in enumerate(h2_steps):
                sA = ppS2.tile([P, 1024], f32, tag="s2", name=f"tA{rp}_{mp}")
                sB = ppS2.tile([P, 1024], f32, tag="s2", name=f"tB{rp}_{mp}")
                for q2 in range(2):
                    m = 2 * mp + q2
                    for ri, s_t in ((0, sA), (1, sB)):
                        sq = slice(512 * (2 * rp + ri), 512 * (2 * rp + ri + 1))
                        nc.tensor.matmul(
                            s_t[:, 512 * q2 : 512 * (q2 + 1)],
                            kTb[0:DH, P * m : P * (m + 1)],
                            qTb[0:DH, sq],
                            start=True, stop=True,
                        )
                ptA = ppool.tile([P, 1024], bf16, tag="p", bufs=6, name="ptA")
                nc.scalar.activation(ptA[:], sA[:], Act.Exp, scale=SCALE)
                ptB = ppool.tile([P, 1024], bf16, tag="p", bufs=6, name="ptB")
                nc.scalar.activation(ptB[:], sB[:], Act.Exp, scale=SCALE)
                pt2s[si] = (ptA, ptB)
                if si >= 1:
                    h2_avs(si - 1)
                if si == NM // 2:
                    # rp=0 accumulation just stopped: divide + ship r=0,1 now
                    for r_e in (0, 1):
                        evict_head(2, r_e, po2s[r_e])
                        for g2 in range(2):
                            nc.sync.dma_start(
                                a2a2_in[NJ * g2 + r_e, :, :],
                                o_hb[2][:, 512 * r_e : 512 * (r_e + 1)],
                            )
                    # prefetch + mask the heads-0,1 exchange result so the
                    # output projection starts the instant head 2 finishes
                    oGt1 = rows.tile([P, NJ, QS], bf16, tag="krow", bufs=6, name="oGt1")
                    nc.sync.dma_start(
                        oG[:, 0:NJ, :],
                        a2a1_out[0:NJ, :, :].rearrange("r s w -> s r w"),
                    )
                    nc.sync.dma_start(
                        oGt1[:],
                        a2a1_out[NJ : 2 * NJ, :, :].rearrange("r s w -> s r w"),
                    )
                    nc.vector.tensor_scalar(
                        oG[:, 0:NJ, :], oG[:, 0:NJ, :], s0b[:], None, Alu.mult
                    )
                    nc.vector.tensor_scalar(oGt1[:], oGt1[:], s1b[:], None, Alu.mult)
                    nc.vector.tensor_tensor(
                        oG[:, 0:NJ, :], oG[:, 0:NJ, :], oGt1[:], Alu.add
                    )
            h2_avs(len(h2_steps) - 1)
            for r in (2, 3):
                evict_head(2, r, po2s[r])
            for r in (2, 3):
                for g2 in range(2):
                    nc.sync.dma_start(
                        a2a2_in[NJ * g2 + r, :, :],
                        o_hb[2][:, 512 * r : 512 * (r + 1)],
                    )

        nc.gpsimd.collective_compute(
            "AllToAll",
            Alu.bypass,
            replica_groups=[list(range(NCORES))],
            ins=[a2a2_in.opt()],
            outs=[a2a2_out.opt()],
        )

        # ========== output projection + residual + LayerNorm ==========
        with tc.tile_pool(name="ppD", bufs=4, space="PSUM") as ppD:
            px = {}
            def d_partial(qt, ci_list, start_first):
                if qt not in px:
                    px[qt] = ppD.tile([P, C], f32, tag="px", name=f"px{qt}")
                for idx, ci in enumerate(ci_list):
                    st = dict(start=(start_first and idx == 0), stop=(ci == CI - 1))
                    nc.tensor.matmul(
                        px[qt][:, 0:512],
                        oG[:, ci, P * qt : P * (qt + 1)],
                        wo_sb[:, ci, 0:512],
                        **st,
                    )
                    nc.tensor.matmul(
                        px[qt][:, 512:C],
                        oG[:, ci, P * qt : P * (qt + 1)],
                        wo_sb[:, ci, 512:C],
                        **st,
                    )

            # out-proj over the already-received 2/3 of the contraction
            # (overlaps the head-2 AllToAll)
            for qt in range(QT):
                d_partial(qt, [0, 1, 2, 3], True)

            # oG ci 4,5 from the head-2 exchange
            for r2 in range(2):
                nc.sync.dma_start(
                    oG[:, 4:6, :].rearrange("(r2 s) o w -> r2 s o w", s=DH)[r2],
                    a2a2_out[0:NJ, :, :].rearrange("(o r2) s w -> r2 s o w", r2=2)[r2],
                )
            oGt2 = rows.tile([P, 2, QS], bf16, tag="ogt2", bufs=1, name="oGt2")
            for r2 in range(2):
                nc.sync.dma_start(
                    oGt2[:].rearrange("(r2 s) o w -> r2 s o w", s=DH)[r2],
                    a2a2_out[NJ : 2 * NJ, :, :].rearrange(
                        "(o r2) s w -> r2 s o w", r2=2
                    )[r2],
                )
            nc.vector.tensor_scalar(
                oG[:, 4:6, :], oG[:, 4:6, :], s0b[:], None, Alu.mult
            )
            nc.vector.tensor_scalar(oGt2[:], oGt2[:], s1b[:], None, Alu.mult)
            nc.vector.tensor_tensor(oG[:, 4:6, :], oG[:, 4:6, :], oGt2[:], Alu.add)

            for qt in range(QT):
                d_partial(qt, [4, 5], False)
                # x1 = out-proj + residual (qres has bo folded in host-side)
                x1 = ppool.tile([P, C], f32, tag="x1", bufs=2, name="x1")
                nc.vector.tensor_tensor(x1[:], px[qt][:], qres_sb[:, qt], Alu.add)
                musum = small.tile([P, 1], f32, tag="st", name="musum")
                mu = small.tile([P, 1], f32, tag="st", name="mu")
                sq_t = ppool.tile([P, C], f32, tag="sq", bufs=2, name="sq")
                sqs = small.tile([P, 1], f32, tag="st", name="sqs")
                var = small.tile([P, 1], f32, tag="st", name="var")
                rinv = small.tile([P, 1], f32, tag="st", name="rinv")
                rstd = small.tile([P, 1], f32, tag="st", name="rstd")
                nb = small.tile([P, 1], f32, tag="st", name="nb")
                nc.vector.reduce_sum(musum[:], x1[:], axis=mybir.AxisListType.X)
                nc.vector.tensor_scalar_mul(mu[:], musum[:], 1.0 / C)
                nc.scalar.activation(sq_t[:], x1[:], Act.Square, accum_out=sqs[:])
                nc.vector.tensor_tensor(var[:], mu[:], mu[:], Alu.mult)
                nc.vector.scalar_tensor_tensor(
                    var[:], sqs[:], 1.0 / C, var[:], Alu.mult, Alu.subtract
                )
                nc.vector.tensor_scalar_add(var[:], var[:], EPS)
                nc.vector.reciprocal(rinv[:], var[:])
                nc.scalar.activation(rstd[:], rinv[:], Act.Sqrt)
                nc.vector.scalar_tensor_tensor(
                    nb[:], mu[:], -1.0, rstd[:], Alu.mult, Alu.mult
                )
                nc.vector.tensor_scalar(
                    x1[:], x1[:], rstd[:], nb[:], Alu.mult, Alu.add
                )
                nc.gpsimd.tensor_tensor(x1[:], x1[:], gamma_b[:], Alu.mult)
                nc.vector.tensor_tensor(x1[:], x1[:], beta_b[:], Alu.add)
                nc.sync.dma_start(
                    y.rearrange("(t p) c -> p t c", p=P)[:, qt], x1[:]
                )
                del px[qt]

        for pool in (dram, small, ppool, rows, persist, const):
            pool.release()

    nc.compile()
    return nc


def get_nc():
    if "nc" not in _NC_CACHE:
        _NC_CACHE["nc"] = _build_nc()
    return _NC_CACHE["nc"]


def make_in_maps(inputs):
    import ml_dtypes

    b16 = ml_dtypes.bfloat16
    q = np.asarray(inputs["query"], np.float32)
    k = np.asarray(inputs["key_in"], np.float32)
    v = np.asarray(inputs["value"], np.float32)
    Wq = np.asarray(inputs["Wq"], np.float32)
    Wk = np.asarray(inputs["Wk"], np.float32)
    Wv = np.asarray(inputs["Wv"], np.float32)
    Wo = np.asarray(inputs["Wo"], np.float32)
    bq = np.asarray(inputs["bq"], np.float32)
    bk = np.asarray(inputs["bk"], np.float32)
    bv = np.asarray(inputs["bv"], np.float32)
    bo = np.asarray(inputs["bo"], np.float32)
    gamma = np.asarray(inputs["gamma"], np.float32)
    beta = np.asarray(inputs["beta"], np.float32)

    in_maps = []
    for c in range(NCORES):
        b, g = c // 4, c % 4
        cs = slice(CS * g, CS * (g + 1))
        in_maps.append(
            {
                "qT": np.ascontiguousarray(q[b].T).astype(b16),
                "kT": np.ascontiguousarray(k[b].T).astype(b16),
                "vT": np.ascontiguousarray(v[b].T).astype(b16),
                "wq": np.ascontiguousarray(Wq[:, cs]).astype(b16),
                "wk": np.ascontiguousarray(Wk[:, cs]).astype(b16),
                "wv": np.ascontiguousarray(Wv[:, cs]).astype(b16),
                "wo": Wo[WO_PERM, :].astype(b16),
                "bq": np.ascontiguousarray(bq[cs]),
                "bk": np.ascontiguousarray(bk[cs]),
                "bv": np.ascontiguousarray(bv[cs]),
                "gamma": gamma.copy(),
                "beta": beta.copy(),
                "qres": np.ascontiguousarray(q[b, QS * g : QS * (g + 1)] + bo[None, :]),
                "gsel": np.array([1.0 - b, float(b)], np.float32),
            }
        )
    return in_maps


def _install_ntff_shim():
    """Provide antenv.axon_hooks if the image lacks it (needed for trace=True)."""
    try:
        import antenv.axon_hooks  # noqa: F401

        return
    except ImportError:
        pass
    import contextlib
    import ctypes
    import types

    so_path = "/opt/axon/libaxon_pjrt.so"
    state = {"hook": None}

    def set_axon_ntff_profile_hook(h):
        state["hook"] = h

    def get_axon_ntff_profile_hook():
        if state["hook"] is None:
            try:
                lib = ctypes.CDLL(so_path)
            except OSError:
                return None
            if not hasattr(lib, "axon_start_nrt_profile"):
                return None
            lib.axon_start_nrt_profile.argtypes = [
                ctypes.POINTER(ctypes.c_int64),
                ctypes.c_size_t,
            ]
            lib.axon_start_nrt_profile.restype = ctypes.c_int64
            lib.axon_stop_nrt_profile.argtypes = [ctypes.c_char_p]
            lib.axon_stop_nrt_profile.restype = ctypes.c_int64

            @contextlib.contextmanager
            def _hook(output_dir, device_ids):
                import jax

                jax.devices()
                if device_ids:
                    ids = (ctypes.c_int64 * len(device_ids))(*device_ids)
                    rc = lib.axon_start_nrt_profile(ids, len(device_ids))
                else:
                    rc = lib.axon_start_nrt_profile(None, 0)
                if rc != 0:
                    raise RuntimeError(f"axon_start_nrt_profile rc={rc}")
                try:
                    yield
                finally:
                    n = lib.axon_stop_nrt_profile(str(output_dir).encode())
                    print(f"profile: {n} file(s) written to {output_dir}")

            state["hook"] = _hook
        return state["hook"]

    mod = types.ModuleType("antenv.axon_hooks")
    mod.set_axon_ntff_profile_hook = set_axon_ntff_profile_hook
    mod.get_axon_ntff_profile_hook = get_axon_ntff_profile_hook
    import antenv

    antenv.axon_hooks = mod
    sys.modules["antenv.axon_hooks"] = mod


def run(inputs, trace=False, trace_cores=None):
    if trace:
        _install_ntff_shim()
    from concourse.bass_utils import run_bass_kernel_spmd

    nc = get_nc()
    in_maps = make_in_maps(inputs)
    res = run_bass_kernel_spmd(
        nc,
        in_maps,
        list(range(NCORES)),
        trace=trace,
        **({"trace_cores": trace_cores} if trace_cores is not None else {}),
    )
    out = np.empty((B, N, C), np.float32)
    for c in range(NCORES):
        b, g = c // 4, c % 4
        out[b, QS * g : QS * (g + 1)] = res.results[c]["y"]
    return out, res


def kernel(**inputs):
    out, _ = run(inputs, trace=False)
    return out
